# revision 5
# baseline (speedup 1.0000x reference)
"""Trainium2 Bass kernel for nn_LoraQKV (MLA-style LoRA QKV + causal SDPA + o_proj).

Strategy (8 NeuronCores, single NEFF, bf16 matmuls / fp32 PSUM):
  Phase 1 (sequence-sharded): each core computes its 256-token slice of both
    LoRA down-projections, RMSNorm (norm weights folded into the up-proj
    weights host-side), RoPE on the shared k_rope, transposes to
    feature-major bf16, packs, and AllGathers (kv, q, rope as three
    collectives) so every core holds the full-sequence latents.
  Phase 2 (tensor-parallel over heads, 4 heads/core): up-projections emitted
    directly in transposed layout (kT, qT) plus v in natural layout, RoPE on
    q folded into the PSUM->SBUF epilogue, causal attention in scoresT
    layout (exp without max-subtraction -- scores are small by construction).
    Softmax denominators are accumulated on the vector engine (sum of exp
    tiles) and turned into a broadcast reciprocal via a single all-ones
    matmul (fuses partition-sum + broadcast). o_proj is interleaved with the
    attention stream; outputs staged per 128-row block and DMAd on
    alternating queues.
  Host sums the 8 partial [2048, 4096] outputs.

Performance notes:
  - The PE array p-states up to 2.4 GHz only after ~3us of gap-free
    execution; the whole emission order is arranged to keep the tensor
    engine continuously fed.
  - Weights / latents move in few large DMAs (partition-major DRAM layouts)
    spread across the sync/scalar (HWDGE) and gpsimd (SWDGE) queues.

Platform workarounds (this walrus build):
  - at most ONE sync-wait per instruction: extra waits are split onto
    standalone EventSemaphore instructions just before lowering.
  - matmul operands must share a partition base, and the base must stay
    constant within a PSUM accumulation group -> everything lives at
    partition base 0.
"""

import sys

sys.path.insert(0, "/opt/trn_rl_repo")

import numpy as np
import ml_dtypes

import bass_rust
import concourse.bass as bass
import concourse.mybir as mybir
import concourse.tile as tile
from concourse.tile import ScopedClock

F32 = mybir.dt.float32
BF16 = mybir.dt.bfloat16

# ---- problem constants (hardcoded per contract) ----
B, S, HID = 1, 2048, 4096
H, HD, ROPE = 32, 128, 64
QR, KVR = 1536, 896
EPS = 1e-6
SCALE = (HD + ROPE) ** -0.5
NCORES = 8
SS = S // NCORES          # 256 tokens per core in phase 1
HPC = H // NCORES         # 4 heads per core in phase 2
KVW = KVR + ROPE          # 960
KC = HID // 128           # 32 k-chunks
QB = QR // 128            # 12
KB = KVR // 128           # 7

# ============================================================
# walrus single-wait workaround
# ============================================================

def _mk_wait(name, engine, wait, debug):
    ev = bass_rust.InstEventSemaphore(name=name, ins=[], outs=[])
    ev.engine = engine
    ev.sync_info = mybir.SyncInfo(on_wait=[wait], on_update=[])
    if debug is not None:
        ev.debug = debug
    return ev


def _split_list(insts):
    out = []
    for inst in insts:
        si = getattr(inst, "sync_info", None)
        ow = list(si.on_wait) if si is not None and si.on_wait else []
        if len(ow) > 1:
            for j, w in enumerate(ow[:-1]):
                out.append(_mk_wait(f"{inst.name}-sw{j}", inst.engine, w,
                                    getattr(inst, "debug", None)))
            inst.sync_info = mybir.SyncInfo(on_wait=[ow[-1]],
                                            on_update=list(si.on_update or []))
        out.append(inst)
    return out


_PATCHED = False


def _install_tile_patches():
    global _PATCHED
    if _PATCHED:
        return
    _PATCHED = True

    _orig_lower = tile.TileContext._lower_ordered_insts

    def _lower_split(self, ordered):
        ordered = {bb: _split_list(insts) for bb, insts in ordered.items()}
        return _orig_lower(self, ordered)

    tile.TileContext._lower_ordered_insts = _lower_split

    def _drain_and_barrier(self, tick_clock, wait_clock):
        nc = self.nc
        probe = nc.sync.nop(nofuse=True)
        wait_clock.add_sem_waits(probe.ins,
                                 ScopedClock({None: tick_clock.global_clock}))
        waits = list(probe.ins.sync_info.on_wait or [])
        probe.ins.sync_info = mybir.SyncInfo(on_wait=waits[:1], on_update=[])
        for w in waits[1:]:
            n = nc.sync.nop(nofuse=True)
            n.ins.sync_info = mybir.SyncInfo(on_wait=[w], on_update=[])
        nc.sync.drain()
        nc.all_engine_barrier()
        assert self.sems is not None
        popped = nc._tile_sem_poison_stack.pop()
        assert popped is self._sem_poison
        nc.clear_and_free_semaphores(list(self.sems.allocated().values()))
        nc.all_engine_barrier()

    tile.TileContext._drain_and_barrier = _drain_and_barrier


# ============================================================
# kernel builder
# ============================================================

def build_nc():
    _install_tile_patches()
    nc = bass.Bass()

    # ---- external inputs (per-core shards prepared host-side) ----
    xT = nc.declare_dram_parameter("xT", [128, KC, SS], BF16, isOutput=False)
    wkvT = nc.declare_dram_parameter("wkvT", [128, KC, KVW], BF16, isOutput=False)
    wqT = nc.declare_dram_parameter("wqT", [128, KC, QR], BF16, isOutput=False)
    cos_sh = nc.declare_dram_parameter("cos_sh", [2, 128, ROPE], F32, isOutput=False)
    sin_sh = nc.declare_dram_parameter("sin_sh", [2, 128, ROPE], F32, isOutput=False)
    qbhiT = nc.declare_dram_parameter("qbhiT", [QB, 128, HPC, 128], BF16, isOutput=False)
    qbloT = nc.declare_dram_parameter("qbloT", [QB, 128, HPC // 2, 128], BF16, isOutput=False)
    kvbkT = nc.declare_dram_parameter("kvbkT", [KB, 128, HPC, 128], BF16, isOutput=False)
    kvbvT = nc.declare_dram_parameter("kvbvT", [KB, 128, HPC * 128], BF16, isOutput=False)
    owT = nc.declare_dram_parameter("owT", [HPC, 128, HID], BF16, isOutput=False)
    cosT_s = nc.declare_dram_parameter("cosT_s", [ROPE, S], F32, isOutput=False)   # cos.T * SCALE
    sinTn_s = nc.declare_dram_parameter("sinTn_s", [ROPE, S], F32, isOutput=False) # signed sin.T * SCALE
    ident = nc.declare_dram_parameter("ident", [128, 128], BF16, isOutput=False)
    onesb = nc.declare_dram_parameter("onesb", [128, 128], BF16, isOutput=False)
    cmask = nc.declare_dram_parameter("cmask", [4, 128, 512], BF16, isOutput=False)  # -30000 where masked

    o_part = nc.declare_dram_parameter("o_part", [S, HID], BF16, isOutput=True)

    pack_kv = nc.dram_tensor("pack_kv", [KVR, SS], BF16)
    pack_rp = nc.dram_tensor("pack_rp", [ROPE, SS], BF16)
    pack_q = nc.dram_tensor("pack_q", [QR, SS], BF16)
    gath_kv = nc.dram_tensor("gath_kv", [NCORES * KVR, SS], BF16, addr_space="Shared")
    gath_rp = nc.dram_tensor("gath_rp", [NCORES * ROPE, SS], BF16, addr_space="Shared")
    gath_q = nc.dram_tensor("gath_q", [NCORES * QR, SS], BF16, addr_space="Shared")

    GK = 8                      # k-chunks per weight DMA group
    NG = KC // GK               # 4 groups

    with tile.TileContext(nc) as tc:
        # ---- two-sided SBUF pool layout (strict LIFO per side) ----
        cpool_ctx = tc.tile_pool(name="const", bufs=1, side="left")
        cpool = cpool_ctx.__enter__()
        qkvp_ctx = tc.tile_pool(name="qkv", bufs=1, side="left")
        qkvp = qkvp_ctx.__enter__()
        wkvw_ctx = tc.tile_pool(name="wear_kv", bufs=1, side="left")
        wkvw = wkvw_ctx.__enter__()
        p1_ctx = tc.tile_pool(name="p1", bufs=1, side="right")
        p1 = p1_ctx.__enter__()

        # phase-1-critical load first on the sync HWDGE queue
        xT_sb = p1.tile([128, KC, SS], BF16)
        nc.sync.dma_start(xT_sb[:], xT.ap())

        # constants / kv up-proj weights prefetch on the SWDGE queue
        id_sb = cpool.tile([128, 128], BF16)
        nc.gpsimd.dma_start(id_sb[:], ident[:])
        cos_sb = cpool.tile([128, 2, ROPE], F32)
        nc.gpsimd.dma_start(cos_sb[:], cos_sh.ap().rearrange("b p r -> p b r"))
        sin_sb = cpool.tile([128, 2, ROPE], F32)
        nc.gpsimd.dma_start(sin_sb[:], sin_sh.ap().rearrange("b p r -> p b r"))
        ones_sb = cpool.tile([128, 128], BF16)
        nc.gpsimd.dma_start(ones_sb[:], onesb[:])
        cmask_sb = cpool.tile([128, 4, 512], BF16)
        nc.gpsimd.dma_start(cmask_sb[:], cmask.ap().rearrange("m p c -> p m c"))
        kvbk_sb = wkvw.tile([128, KB, HPC, 128], BF16)
        nc.gpsimd.dma_start(kvbk_sb[:], kvbkT.ap().rearrange("c p h d -> p c h d"))
        kvbv_sb = wkvw.tile([128, KB, HPC * 128], BF16)
        nc.gpsimd.dma_start(kvbv_sb[:], kvbvT.ap().rearrange("c p d -> p c d"))

        # ================= phase 1: down-proj on this core's 256 tokens ========
        pack_kv_sb = p1.tile([128, KB, SS], BF16)
        pack_rp_sb = p1.tile([ROPE, SS], BF16)
        pack_q_sb = p1.tile([128, QB, SS], BF16)

        with tc.tile_pool(name="p1_ps", bufs=1, space="PSUM") as psp, \
             tc.tile_pool(name="p1_pst", bufs=2, space="PSUM") as psp_t, \
             tc.tile_pool(name="p1_tmp", bufs=2, side="right") as tpool:

            # -- pass A: kv_lat + rope; 4 psum banks
            with tc.tile_pool(name="p1_wkv", bufs=2, side="right") as wkvp:
                pskv = [[psp.tile([128, 512], F32, name=f"pskv{sb}{nt}",
                                  tag=f"ps_big_{sb * 3 + nt}") for nt in range(2)]
                        for sb in range(2)]
                for g in range(NG):
                    wkv = wkvp.tile([128, GK, KVW], BF16, tag="wkv")
                    eng = nc.scalar if g % 2 == 0 else nc.sync
                    eng.dma_start(wkv[:], wkvT.ap()[:, g * GK:(g + 1) * GK, :])
                    for kk in range(GK):
                        k = g * GK + kk
                        st, sp = (k == 0), (k == KC - 1)
                        for sb in range(2):
                            lhs = xT_sb[:, k, sb * 128:(sb + 1) * 128]
                            nc.tensor.matmul(pskv[sb][0][:], lhs,
                                             wkv[:, kk, 0:512], start=st, stop=sp)
                            nc.tensor.matmul(pskv[sb][1][:, 0:KVW - 512], lhs,
                                             wkv[:, kk, 512:KVW], start=st, stop=sp)
                for sb in range(2):
                    sqt = tpool.tile([128, 512], F32, tag="sqt")
                    nt8 = tpool.tile([128, 8], F32, tag="nt8")
                    nc.scalar.activation(sqt[:], pskv[sb][0][:],
                                         mybir.ActivationFunctionType.Square,
                                         accum_out=nt8[:, 0:1])
                    nc.scalar.activation(sqt[:, 0:KVR - 512], pskv[sb][1][:, 0:KVR - 512],
                                         mybir.ActivationFunctionType.Square,
                                         accum_out=nt8[:, 1:2])
                    nc.vector.reduce_sum(nt8[:, 4:5], nt8[:, 0:2], axis=mybir.AxisListType.X)
                    nc.vector.tensor_scalar(nt8[:, 5:6], nt8[:, 4:5], 1.0 / KVR, EPS,
                                            mybir.AluOpType.mult, mybir.AluOpType.add)
                    nc.scalar.activation(nt8[:, 5:6], nt8[:, 5:6],
                                         mybir.ActivationFunctionType.Sqrt)
                    nc.vector.reciprocal(nt8[:, 6:7], nt8[:, 5:6])
                    kvn = tpool.tile([128, KVR], BF16, tag="kvn")
                    nc.vector.tensor_scalar_mul(kvn[:, 0:512], pskv[sb][0][:], nt8[:, 6:7])
                    nc.vector.tensor_scalar_mul(kvn[:, 512:KVR], pskv[sb][1][:, 0:KVR - 512],
                                                nt8[:, 6:7])
                    for rc in range(KB):
                        pst = psp_t.tile([128, 128], BF16, tag="pst")
                        nc.tensor.transpose(pst[:], kvn[:, rc * 128:(rc + 1) * 128], id_sb[:])
                        nc.scalar.copy(pack_kv_sb[:, rc, sb * 128:(sb + 1) * 128], pst[:])
                    # rope on k_rope = pskv[sb][1][:, 384:448]
                    RP = KVR - 512  # 384
                    t1 = tpool.tile([128, ROPE], F32, tag="ropet1")
                    t2 = tpool.tile([128, ROPE], F32, tag="ropet2")
                    nc.vector.tensor_mul(t1[:], pskv[sb][1][:, RP:RP + ROPE], cos_sb[:, sb, :])
                    nc.vector.tensor_mul(t2[:, 0:32], pskv[sb][1][:, RP + 32:RP + 64],
                                         sin_sb[:, sb, 0:32])
                    nc.vector.tensor_sub(t1[:, 0:32], t1[:, 0:32], t2[:, 0:32])
                    nc.vector.tensor_mul(t2[:, 32:64], pskv[sb][1][:, RP:RP + 32],
                                         sin_sb[:, sb, 32:64])
                    nc.vector.tensor_add(t1[:, 32:64], t1[:, 32:64], t2[:, 32:64])
                    kr = tpool.tile([128, ROPE], BF16, tag="kr")
                    nc.vector.tensor_copy(kr[:], t1[:])
                    pst = psp_t.tile([128, 128], BF16, tag="pst")
                    nc.tensor.transpose(pst[0:ROPE, :], kr[:], id_sb[:])
                    nc.scalar.copy(pack_rp_sb[:, sb * 128:(sb + 1) * 128], pst[0:ROPE, :])
            nc.gpsimd.dma_start(pack_kv.ap().rearrange("(c p) s -> p c s", p=128),
                                pack_kv_sb[:])
            nc.gpsimd.dma_start(pack_rp.ap(), pack_rp_sb[:])
            nc.gpsimd.collective_compute(
                "AllGather", mybir.AluOpType.bypass,
                replica_groups=[list(range(NCORES))],
                ins=[pack_kv.ap().opt()],
                outs=[gath_kv.ap().opt()],
            )

            # -- pass B: q_lat; 6 psum banks
            with tc.tile_pool(name="p1_wq", bufs=2, side="right") as wqp:
                psq = [[psp.tile([128, 512], F32, name=f"psq{sb}{nt}",
                                 tag=f"ps_big_{sb * 3 + nt}") for nt in range(3)]
                       for sb in range(2)]
                for g in range(NG):
                    wq = wqp.tile([128, GK, QR], BF16, tag="wq")
                    eng = nc.scalar if g % 2 == 0 else nc.sync
                    eng.dma_start(wq[:], wqT.ap()[:, g * GK:(g + 1) * GK, :])
                    for kk in range(GK):
                        k = g * GK + kk
                        st, sp = (k == 0), (k == KC - 1)
                        for sb in range(2):
                            lhs = xT_sb[:, k, sb * 128:(sb + 1) * 128]
                            for nt in range(3):
                                nc.tensor.matmul(psq[sb][nt][:], lhs,
                                                 wq[:, kk, nt * 512:(nt + 1) * 512],
                                                 start=st, stop=sp)
                for sb in range(2):
                    sqt = tpool.tile([128, 512], F32, tag="sqt")
                    nt8 = tpool.tile([128, 8], F32, tag="nt8")
                    for nt in range(3):
                        nc.scalar.activation(sqt[:], psq[sb][nt][:],
                                             mybir.ActivationFunctionType.Square,
                                             accum_out=nt8[:, nt:nt + 1])
                    nc.vector.reduce_sum(nt8[:, 4:5], nt8[:, 0:3], axis=mybir.AxisListType.X)
                    nc.vector.tensor_scalar(nt8[:, 5:6], nt8[:, 4:5], 1.0 / QR, EPS,
                                            mybir.AluOpType.mult, mybir.AluOpType.add)
                    nc.scalar.activation(nt8[:, 5:6], nt8[:, 5:6],
                                         mybir.ActivationFunctionType.Sqrt)
                    nc.vector.reciprocal(nt8[:, 6:7], nt8[:, 5:6])
                    qn = tpool.tile([128, QR], BF16, tag="qn")
                    for nt in range(3):
                        nc.vector.tensor_scalar_mul(qn[:, nt * 512:(nt + 1) * 512],
                                                    psq[sb][nt][:], nt8[:, 6:7])
                    for rc in range(QB):
                        pst = psp_t.tile([128, 128], BF16, tag="pst")
                        nc.tensor.transpose(pst[:], qn[:, rc * 128:(rc + 1) * 128], id_sb[:])
                        nc.scalar.copy(pack_q_sb[:, rc, sb * 128:(sb + 1) * 128], pst[:])
            nc.gpsimd.dma_start(pack_q.ap().rearrange("(c p) s -> p c s", p=128),
                                pack_q_sb[:])
            nc.gpsimd.collective_compute(
                "AllGather", mybir.AluOpType.bypass,
                replica_groups=[list(range(NCORES))],
                ins=[pack_q.ap().opt()],
                outs=[gath_q.ap().opt()],
            )
            nc.gpsimd.collective_compute(
                "AllGather", mybir.AluOpType.bypass,
                replica_groups=[list(range(NCORES))],
                ins=[pack_rp.ap().opt()],
                outs=[gath_rp.ap().opt()],
            )
        p1_ctx.__exit__(None, None, None)

        # ================= phase 2 =================
        ropeT = qkvp.tile([ROPE, NCORES, SS], BF16)
        k_nope = [qkvp.tile([128, 16, 128], BF16, name=f"k_nope_{h}") for h in range(HPC)]
        v_sb = qkvp.tile([128, 16, HPC, 128], BF16)
        q_rope = [qkvp.tile([ROPE, 4, 512], BF16, name=f"q_rope_{h}") for h in range(HPC)]
        q_nope = [qkvp.tile([128, 4, 512], BF16, name=f"q_nope_{h}") for h in range(HPC)]

        # ---- latent reloads (big consolidated DMAs) + q-up weight prefetch ----
        latq_ctx = tc.tile_pool(name="latq", bufs=1, side="right")
        latq = latq_ctx.__enter__()
        wqw_ctx = tc.tile_pool(name="wear_q", bufs=1, side="right")
        wqw = wqw_ctx.__enter__()
        latkv_ctx = tc.tile_pool(name="latkv", bufs=1, side="right")
        latkv = latkv_ctx.__enter__()

        kvlatT = latkv.tile([128, KB, NCORES, SS], BF16)
        gkv = gath_kv.ap().rearrange("(b c p) s -> p c b s", b=NCORES, c=KB)
        for rc in range(KB):
            eng = nc.sync if rc % 2 == 0 else nc.scalar
            eng.dma_start(kvlatT[:, rc], gkv[:, rc])
        nc.gpsimd.dma_start(ropeT[:],
                            gath_rp.ap().rearrange("(b p) s -> p b s", b=NCORES))
        qlat = latq.tile([128, QB, NCORES, SS], BF16)
        gq = gath_q.ap().rearrange("(b c p) s -> p c b s", b=NCORES, c=QB)
        for rc in range(QB):
            eng = nc.sync if rc % 2 == 0 else nc.scalar
            eng.dma_start(qlat[:, rc], gq[:, rc])
        qbhi_sb = wqw.tile([128, QB, HPC, 128], BF16)
        nc.gpsimd.dma_start(qbhi_sb[:], qbhiT.ap().rearrange("c p h d -> p c h d"))
        qblo_sb = wqw.tile([128, QB, HPC // 2, 128], BF16)
        nc.gpsimd.dma_start(qblo_sb[:], qbloT.ap().rearrange("c p h d -> p c h d"))
        cosT_sb = wqw.tile([ROPE, S], F32)
        nc.gpsimd.dma_start(cosT_sb[:], cosT_s[:])
        sinT_sb = wqw.tile([ROPE, S], F32)
        nc.gpsimd.dma_start(sinT_sb[:], sinTn_s[:])

        # ---- kv up-proj (overlaps q AllGather) ----
        with tc.tile_pool(name="p2_pskv", bufs=2, space="PSUM") as pskvp:
            for h in range(HPC):
                for st in range(4):
                    ps_k = pskvp.tile([128, 512], F32, tag="ps_k")
                    for rc in range(KB):
                        nc.tensor.matmul(ps_k[:], kvbk_sb[:, rc, h, :],
                                         kvlatT[:, rc, 2 * st:2 * st + 2, :],
                                         start=(rc == 0), stop=(rc == KB - 1))
                    nc.vector.tensor_copy(k_nope[h][:, st * 4:(st + 1) * 4, :], ps_k[:])
                for sk in range(4 * h, 4 * h + 4):
                    ps_v = pskvp.tile([128, 512], F32, tag="ps_v")
                    for rc in range(KB):
                        nc.tensor.matmul(ps_v[:],
                                         kvlatT[:, rc, sk // 2,
                                                (sk % 2) * 128:(sk % 2) * 128 + 128],
                                         kvbv_sb[:, rc, :],
                                         start=(rc == 0), stop=(rc == KB - 1))
                    nc.scalar.copy(v_sb[:, sk, :, :], ps_v[:])
        latkv_ctx.__exit__(None, None, None)
        wkvw_ctx.__exit__(None, None, None)

        # ---- q up-proj: qt descending so attention J=3 deps land first ----
        with tc.tile_pool(name="p2_psup", bufs=1, space="PSUM") as psup, \
             tc.tile_pool(name="p2_qtmp", bufs=2, side="right") as tq:
            for qt in (3, 2, 1, 0):
                ps_hi = [psup.tile([128, 512], F32, name=f"ps_hi{h}", tag=f"ps_hi{h}")
                         for h in range(HPC)]
                ps_lo = [psup.tile([128, 512], F32, name=f"ps_lo{p}", tag=f"ps_lo{p}")
                         for p in range(HPC // 2)]
                for rc in range(QB):
                    st, sp = (rc == 0), (rc == QB - 1)
                    rhs = qlat[:, rc, 2 * qt:2 * qt + 2, :]
                    for h in range(HPC):
                        nc.tensor.matmul(ps_hi[h][:], qbhi_sb[:, rc, h, :],
                                         rhs, start=st, stop=sp)
                    for p in range(HPC // 2):
                        nc.tensor.matmul(ps_lo[p][:], qblo_sb[:, rc, p, :],
                                         rhs, start=st, stop=sp)
                for h in range(HPC):
                    pr, i = h // 2, h % 2
                    nc.scalar.activation(q_nope[h][0:64, qt, :], ps_hi[h][64:128, :],
                                         mybir.ActivationFunctionType.Copy, scale=SCALE)
                    nc.scalar.activation(q_nope[h][64:128, qt, :],
                                         ps_lo[pr][i * 64:(i + 1) * 64, :],
                                         mybir.ActivationFunctionType.Copy, scale=SCALE)
                    stg = tq.tile([ROPE, 512], F32, tag="stg")
                    nc.scalar.copy(stg[:], ps_hi[h][0:ROPE, :])
                    rot = tq.tile([ROPE, 512], F32, tag="rot")
                    nc.scalar.dma_start(rot[0:32, :], stg[32:64, :])
                    nc.scalar.dma_start(rot[32:64, :], stg[0:32, :])
                    qsl = slice(qt * 512, (qt + 1) * 512)
                    m1 = tq.tile([ROPE, 512], F32, tag="m1")
                    nc.vector.tensor_mul(m1[:], stg[:], cosT_sb[:, qsl])
                    m2 = tq.tile([ROPE, 512], F32, tag="m2")
                    nc.vector.tensor_mul(m2[:], rot[:], sinT_sb[:, qsl])
                    nc.vector.tensor_add(q_rope[h][:, qt, :], m1[:], m2[:])
        wqw_ctx.__exit__(None, None, None)
        latq_ctx.__exit__(None, None, None)

        # ---- attention-scoped tiles; o_proj weights arrive while J=3 runs ----
        attp_ctx = tc.tile_pool(name="attp", bufs=1, side="left")
        attp = attp_ctx.__enter__()
        attnT = attp.tile([128, HPC, 16, 128], BF16)
        owT_sb = attp.tile([128, HPC, HID], BF16)
        nc.gpsimd.dma_start(owT_sb[:], owT.ap().rearrange("h p d -> p h d"))

        # ---- causal attention (J-outer) with interleaved o_proj ----
        with tc.tile_pool(name="p2_psat", bufs=3, space="PSUM") as psat, \
             tc.tile_pool(name="p2_psA", bufs=2, space="PSUM") as psAp, \
             tc.tile_pool(name="p2_psoo", bufs=2, space="PSUM") as psoop, \
             tc.tile_pool(name="p2_psd", bufs=1, space="PSUM") as psdp, \
             tc.tile_pool(name="p2_exp", bufs=4, side="left") as expp, \
             tc.tile_pool(name="p2_den", bufs=2, side="left") as denp, \
             tc.tile_pool(name="p2_oo", bufs=2, side="left") as oop, \
             tc.tile_pool(name="p2_tmp", bufs=2, side="left") as tp2:

            state = {"stage": None}
            pending = []

            def emit_chunk():
                sblk, ot = pending.pop(0)
                if ot == 0:
                    state["stage"] = oop.tile([128, 8, 512], BF16,
                                              name="oo_stage", tag="oo_stage")
                ps_oo = psoop.tile([128, 512], F32, tag="ps_oo")
                for hh in range(HPC):
                    nc.tensor.matmul(ps_oo[:], attnT[:, hh, sblk, :],
                                     owT_sb[:, hh, ot * 512:(ot + 1) * 512],
                                     start=(hh == 0), stop=(hh == HPC - 1))
                nc.scalar.copy(state["stage"][:, ot, :], ps_oo[:])
                if ot == 7:
                    eng = nc.sync if sblk % 2 == 0 else nc.scalar
                    eng.dma_start(
                        o_part.ap()[sblk * 128:(sblk + 1) * 128, :],
                        state["stage"][:])

            def drain(n):
                for _ in range(n):
                    if pending:
                        emit_chunk()

            for J in (3, 2, 1, 0):
                nsk = 4 * J + 4
                for h in range(HPC):
                    ps_A = psAp.tile([128, 512], F32, name="ps_A", tag="ps_A")
                    den = denp.tile([128, 512], F32, tag="den")
                    prev_expT = None
                    for b in range(nsk):
                        diag = b >= 4 * J
                        m = b - 4 * J if diag else 0
                        col0 = 128 * m if diag else 0
                        ps_s = psat.tile([128, 512], F32, tag="ps_s")
                        if diag:
                            nc.tensor.matmul(ps_s[:], id_sb[:],
                                             cmask_sb[:, m, :],
                                             start=True, stop=False)
                        nc.tensor.matmul(ps_s[:, col0:512],
                                         ropeT[:, b // 2,
                                               (b % 2) * 128:(b % 2) * 128 + 128],
                                         q_rope[h][:, J, col0:512],
                                         start=(not diag), stop=False)
                        nc.tensor.matmul(ps_s[:, col0:512],
                                         k_nope[h][:, b, :],
                                         q_nope[h][:, J, col0:512],
                                         start=False, stop=True)
                        if prev_expT is not None:
                            nc.tensor.matmul(ps_A[:], v_sb[:, b - 1, h, :],
                                             prev_expT[:],
                                             start=(b == 1), stop=False)
                        expT = expp.tile([128, 512], BF16, tag="expT")
                        nc.scalar.activation(expT[:], ps_s[:],
                                             mybir.ActivationFunctionType.Exp)
                        if b == 0:
                            nc.vector.tensor_copy(den[:], expT[:])
                        else:
                            nc.vector.tensor_add(den[:], den[:], expT[:])
                        prev_expT = expT
                        if b % 2 == 1:
                            drain(1)
                    nc.tensor.matmul(ps_A[:], v_sb[:, nsk - 1, h, :], prev_expT[:],
                                     start=False, stop=True)
                    denb = tp2.tile([128, 512], BF16, tag="denb")
                    nc.scalar.copy(denb[:], den[:])
                    ps_d = psdp.tile([128, 512], F32, tag="ps_d")
                    nc.tensor.matmul(ps_d[:], ones_sb[:], denb[:],
                                     start=True, stop=True)
                    rcf = tp2.tile([128, 512], F32, tag="rcf")
                    nc.vector.reciprocal(rcf[:], ps_d[:])
                    nc.vector.tensor_mul(attnT[:, h, J * 4:(J + 1) * 4, :],
                                         ps_A[:], rcf[:])
                    drain(4)
                pending.extend([(sblk, ot) for sblk in range(4 * J, 4 * J + 4)
                                for ot in range(8)])
            while pending:
                emit_chunk()

        attp_ctx.__exit__(None, None, None)
        qkvp_ctx.__exit__(None, None, None)
        cpool_ctx.__exit__(None, None, None)

    return nc


# ============================================================
# host-side wrapper
# ============================================================

_BUILT = {}


def _get_nc():
    if "nc" not in _BUILT:
        _BUILT["nc"] = build_nc()
    return _BUILT["nc"]


def _bf(x):
    return np.ascontiguousarray(x).astype(ml_dtypes.bfloat16)


def prepare_in_maps(hidden_states, cos, sin, q_a_w, q_a_norm_w, q_b_w,
                    kv_a_w, kv_a_norm_w, kv_b_w, o_w):
    hidden_states = np.asarray(hidden_states, dtype=np.float32)
    cos = np.asarray(cos, dtype=np.float32)
    sin = np.asarray(sin, dtype=np.float32)
    q_a_w = np.asarray(q_a_w, dtype=np.float32)
    q_a_norm_w = np.asarray(q_a_norm_w, dtype=np.float32)
    q_b_w = np.asarray(q_b_w, dtype=np.float32)
    kv_a_w = np.asarray(kv_a_w, dtype=np.float32)
    kv_a_norm_w = np.asarray(kv_a_norm_w, dtype=np.float32)
    kv_b_w = np.asarray(kv_b_w, dtype=np.float32)
    o_w = np.asarray(o_w, dtype=np.float32)

    x = hidden_states.reshape(S, HID)
    # partition-major weight layouts: [128, KC, out]
    wkvT_arr = _bf(kv_a_w.T.reshape(KC, 128, KVW).transpose(1, 0, 2))
    wqT_arr = _bf(q_a_w.T.reshape(KC, 128, QR).transpose(1, 0, 2))
    qb = q_b_w * q_a_norm_w[None, :]                       # [H*(ROPE+HD), QR]
    qbTh = qb.reshape(H, ROPE + HD, QR)
    kvb = kv_b_w * kv_a_norm_w[None, :]                    # [H*2*HD, KVR]
    kvbTh = kvb.reshape(H, 2 * HD, KVR)

    cosT_arr = np.ascontiguousarray(cos.T * SCALE).astype(np.float32)      # [64, S]
    sinT = sin.T * SCALE
    sinTn_arr = np.concatenate([-sinT[0:32], sinT[32:64]], axis=0).astype(np.float32)
    identity = np.eye(128, dtype=np.float32).astype(ml_dtypes.bfloat16)
    ones_arr = np.ones((128, 128), dtype=np.float32).astype(ml_dtypes.bfloat16)
    cm = np.zeros((4, 128, 512), np.float32)
    for mm_ in range(4):
        p = np.arange(128)[:, None]
        c = np.arange(512)[None, :]
        cm[mm_] = np.where(p + 128 * mm_ <= c, 0.0, -30000.0)
    cmask_arr = cm.astype(ml_dtypes.bfloat16)

    in_maps = []
    for c in range(NCORES):
        sl = slice(c * SS, (c + 1) * SS)
        xT_c = _bf(x[sl].T.reshape(KC, 128, SS).transpose(1, 0, 2))
        cos_c = np.ascontiguousarray(cos[sl]).reshape(2, 128, ROPE)
        sin_c = np.ascontiguousarray(sin[sl]).reshape(2, 128, ROPE)
        hsl = slice(c * HPC, (c + 1) * HPC)
        qbh = qbTh[hsl]                                  # [4, 192, QR]
        qbhiT_c = _bf(qbh[:, 0:128, :].transpose(2, 0, 1)).reshape(QB, 128, HPC, 128)
        qblo = qbh[:, 128:192, :].reshape(HPC // 2, 128, QR)  # [2, 2*64, QR]
        qbloT_c = _bf(qblo.transpose(2, 0, 1)).reshape(QB, 128, HPC // 2, 128)
        kvbh = kvbTh[hsl]                                # [4, 256, KVR]
        kvbkT_c = _bf(kvbh[:, 0:128, :].transpose(2, 0, 1)).reshape(KB, 128, HPC, 128)
        kvbvT_c = _bf(kvbh[:, 128:256, :].reshape(HPC * 128, KVR).T).reshape(KB, 128, HPC * 128)
        owT_c = _bf(o_w[:, c * HPC * HD:(c + 1) * HPC * HD].T).reshape(HPC, 128, HID)
        in_maps.append({
            "xT": xT_c, "wkvT": wkvT_arr, "wqT": wqT_arr,
            "cos_sh": cos_c, "sin_sh": sin_c,
            "qbhiT": qbhiT_c, "qbloT": qbloT_c, "kvbkT": kvbkT_c,
            "kvbvT": kvbvT_c, "owT": owT_c,
            "cosT_s": cosT_arr, "sinTn_s": sinTn_arr,
            "ident": identity, "onesb": ones_arr, "cmask": cmask_arr,
        })
    return in_maps


def run_on_cores(in_maps, trace=False):
    from concourse.bass_utils import run_bass_kernel_spmd
    nc = _get_nc()
    return run_bass_kernel_spmd(nc, in_maps, core_ids=list(range(NCORES)), trace=trace)


def kernel(**inputs):
    in_maps = prepare_in_maps(**inputs)
    res = run_on_cores(in_maps)
    out = np.zeros((S, HID), np.float64)
    for c in range(NCORES):
        out += res.results[c]["o_part"].astype(np.float64)
    return out.astype(np.float32).reshape(B, S, HID)
